# revision 1
# baseline (speedup 1.0000x reference)
"""Multi-head attention forward, sharded over 8 NeuronCores.

Sharding: batch (2) x head-group (4 groups of 4 heads) = 8 cores.
Each core computes, for its batch b and its 4 heads:
  Q^T/K^T = W^T-slices @ x^T (+bias), V token-major,
  S^T[k,q] = K^T.T@Q^T per k-tile (scores transposed so the key-padding
  mask is a per-partition activation bias and exp output feeds P.V directly),
  P^T = exp(scale*S^T + maskbias),  ctx_aug^T = [V|1]^T.T @ P^T
  (ones column yields softmax denominators for free),
  ctx^T = ctx_aug^T[0:64] * broadcast(1/ctx_aug^T[64])  (rank-1 matmul bcast),
  out_partial = ctx^T.T @ W_o^T-slice  ->  [2048, 1024] fp32.
Host sums the 4 partials per batch and adds out_b.
"""

import os
import sys

if "/opt/trn_rl_repo" not in sys.path:
    sys.path.insert(0, "/opt/trn_rl_repo")

import numpy as np
import ml_dtypes

import concourse.bass as bass
import concourse.mybir as mybir
from concourse import bacc
from concourse.bass import ts, ds
from concourse.tile import TileContext
from concourse import bass_utils

BF16 = mybir.dt.bfloat16
F32 = mybir.dt.float32
F32R = mybir.dt.float32r
EXP = mybir.ActivationFunctionType.Exp
MULT = mybir.AluOpType.mult

N_CORES = 8
S = 2048          # sequence length (one batch per core)
HID = 1024
DH = 256          # head dims per core (4 heads x 64)
D = 64
NEG = -50.0       # additive mask bias (post-scale); exp(-50) ~ 2e-22
KC = 1280         # compacted+padded key length (10 k-tiles); ~1024 unmasked
NKT = KC // 128


def build_program(reps=1):
    nc = bacc.Bacc("TRN2", target_bir_lowering=False, debug=False,
                   num_devices=N_CORES)
    xT = nc.dram_tensor("xT", [HID, S], BF16, kind="ExternalInput").ap()
    xTk = nc.dram_tensor("xTk", [HID, KC], BF16, kind="ExternalInput").ap()
    wqT = nc.dram_tensor("wqT", [HID, DH], BF16, kind="ExternalInput").ap()
    wkT = nc.dram_tensor("wkT", [HID, DH], BF16, kind="ExternalInput").ap()
    wvT = nc.dram_tensor("wvT", [HID, DH], BF16, kind="ExternalInput").ap()
    qkvb = nc.dram_tensor("qkvb", [1, 3 * DH], BF16, kind="ExternalInput").ap()
    woT = nc.dram_tensor("woT", [DH, HID], BF16, kind="ExternalInput").ap()
    maskb = nc.dram_tensor("maskb", [128, NKT], F32, kind="ExternalInput").ap()
    out = nc.dram_tensor("out", [S, HID], F32, kind="ExternalOutput").ap()

    with TileContext(nc) as tc:
        with tc.tile_pool(name="const", bufs=1) as cp:
            wq_sb = cp.tile([128, 8, DH], BF16, name="wq_sb")
            wk_sb = cp.tile([128, 8, DH], BF16, name="wk_sb")
            wv_sb = cp.tile([128, 8, DH], BF16, name="wv_sb")
            nc.sync.dma_start(wq_sb, wqT.rearrange("(c p) m -> p c m", p=128))
            nc.sync.dma_start(wk_sb, wkT.rearrange("(c p) m -> p c m", p=128))
            nc.sync.dma_start(wv_sb, wvT.rearrange("(c p) m -> p c m", p=128))
            wo_sb = cp.tile([128, 2, HID], BF16, name="wo_sb")
            nc.sync.dma_start(wo_sb, woT.rearrange("(c p) o -> p c o", p=128))
            qkvb_sb = cp.tile([1, 3 * DH], BF16, name="qkvb_sb")
            nc.sync.dma_start(qkvb_sb, qkvb)
            maskb_sb = cp.tile([128, NKT], F32, name="maskb_sb")
            nc.sync.dma_start(maskb_sb, maskb)
            ones_bf = cp.tile([1, 2048], BF16, name="ones_bf")
            nc.vector.memset(ones_bf, 1.0)
            ones_tmp = cp.tile([1, 64], F32, name="ones_tmp")
            nc.vector.memset(ones_tmp, 1.0)
            ones_f32 = cp.tile([1, 64], F32R, name="ones_f32")
            with nc.allow_low_precision(reason="f32r ones"):
                nc.vector.tensor_copy(ones_f32, ones_tmp)

            xt_sb = cp.tile([128, 8, S], BF16, name="xt_sb")
            xt_view = xT.rearrange("(c p) t -> c p t", p=128)
            xtk_sb = cp.tile([128, 8, KC], BF16, name="xtk_sb")
            xtk_view = xTk.rearrange("(c p) t -> c p t", p=128)
            for c in range(8):
                nc.sync.dma_start(xt_sb[:, c, :], xt_view[c])
                nc.sync.dma_start(xtk_sb[:, c, :], xtk_view[c])

            q_sb = [cp.tile([128, S], BF16, name=f"q_sb{j}") for j in range(2)]
            k_sb = [cp.tile([128, KC], BF16, name=f"k_sb{j}") for j in range(2)]
            v_sb = [cp.tile([128, NKT, 65], BF16, name=f"v_sb{h}")
                    for h in range(4)]
            ctxT = [cp.tile([128, S], BF16, name=f"ctxT{j}") for j in range(2)]
            for h in range(4):
                nc.vector.memset(v_sb[h][:, :, 64:65], 1.0)

            for _rep in range(reps):
                # ---- QKV projection ----
                with tc.tile_pool(name="psC",
                                  bufs=int(os.environ.get("PSC_BUFS", "5")),
                                  space="PSUM") as psC:
                    if os.environ.get("QKV_OLD_ORDER"):
                        qk_jobs = [(wq_sb, 0, q_sb, 0), (wq_sb, 0, q_sb, 1),
                                   (wk_sb, DH, k_sb, 0), (wk_sb, DH, k_sb, 1)]
                        v_first = False
                    else:
                        # K/Q pair 0 first so attention starts ASAP; V next
                        # (PV consumes V tiles progressively, 2 slots behind);
                        # pair-1 K/Q last, hidden under pair-0 attention.
                        qk_jobs = [(wk_sb, DH, k_sb, 0), (wq_sb, 0, q_sb, 0),
                                   (wk_sb, DH, k_sb, 1), (wq_sb, 0, q_sb, 1)]
                        v_first = False

                    def emit_v():
                        for i in range(NKT):         # V token-major (compact)
                            ps = psC.tile([128, DH], F32, name="ps_v",
                                          tag="qkv")
                            for c in range(8):
                                nc.tensor.matmul(ps,
                                                 lhsT=xtk_sb[:, c, ts(i, 128)],
                                                 rhs=wv_sb[:, c, :],
                                                 start=(c == 0), stop=False)
                            nc.tensor.matmul(ps, lhsT=ones_bf[:, 0:128],
                                             rhs=qkvb_sb[:, ds(2 * DH, DH)],
                                             start=False, stop=True)
                            for h in range(4):
                                if os.environ.get("QKV_DVE"):
                                    nc.vector.tensor_copy(v_sb[h][:, i, 0:64],
                                                          ps[:, ts(h, 64)])
                                else:
                                    nc.scalar.copy(v_sb[h][:, i, 0:64],
                                                   ps[:, ts(h, 64)])

                    if v_first:
                        emit_v()
                    for job_i, (w_sb, boff, dst, j) in enumerate(qk_jobs):
                        if (not v_first) and job_i == 2:
                            emit_v()
                        if True:
                            if boff == 0:        # Q: full token range
                                chunks = [(n * 512, 512) for n in range(4)]
                                src_sb = xt_sb
                            else:                # K: compacted tokens
                                chunks = []
                                off = 0
                                while off < KC:
                                    w = min(512, KC - off)
                                    chunks.append((off, w))
                                    off += w
                                src_sb = xtk_sb
                            pss = [psC.tile([128, 512], F32, name="ps_qkv",
                                            tag="qkv")
                                   for _ in range(len(chunks))]
                            for c in range(8):   # stationary reused over n
                                for n, (off, w) in enumerate(chunks):
                                    nc.tensor.matmul(
                                        pss[n][:, 0:w],
                                        lhsT=w_sb[:, c, ts(j, 128)],
                                        rhs=src_sb[:, c, ds(off, w)],
                                        start=(c == 0), stop=False)
                            for n, (off, w) in enumerate(chunks):
                                nc.tensor.matmul(
                                    pss[n][:, 0:w],
                                    lhsT=qkvb_sb[:, ds(boff + j * 128, 128)],
                                    rhs=ones_bf[:, ds(off, w)],
                                    start=False, stop=True)
                                nc.scalar.copy(dst[j][:, ds(off, w)],
                                               pss[n][:, 0:w])
                    if os.environ.get("QKV_OLD_ORDER"):
                        emit_v()

                # ---- attention + output projection (sw-pipelined) ----
                with tc.tile_pool(name="psS", bufs=2, space="PSUM") as psS, \
                     tc.tile_pool(name="psX", bufs=4, space="PSUM") as psX, \
                     tc.tile_pool(name="ptp",
                              bufs=int(os.environ.get("PT_BUFS", "4"))) as ptp, \
                     tc.tile_pool(name="npool", bufs=2) as npool, \
                     tc.tile_pool(name="outp", bufs=3) as outp:

                    def emit_pv(job):
                        jh, jctx, jpt, jkt = job[:4]
                        for qc in range(2):
                            nc.tensor.matmul(
                                jctx[:, ts(qc, 512)],
                                lhsT=v_sb[jh][:, jkt, :],
                                rhs=jpt[:, ts(qc, 512)],
                                start=(jkt == 0), stop=(jkt == NKT - 1))

                    def emit_norm(job):
                        jqh, jht, jhr, jctx = job[:4]
                        recip = npool.tile([1, 1024], F32R, name="recip")
                        with nc.allow_low_precision(reason="f32r recip"):
                            nc.vector.reciprocal(recip, jctx[64:65, :])
                        rbc = npool.tile([64, 1024], F32, name="rbc")
                        bc_ps = psS.tile([128, 1024], F32, name="s_ps",
                                         tag="s_ps")
                        for qc in range(2):
                            nc.tensor.matmul(bc_ps[0:64, ts(qc, 512)],
                                             lhsT=ones_f32,
                                             rhs=recip[:, ts(qc, 512)],
                                             start=True, stop=True)
                        nc.vector.tensor_copy(rbc, bc_ps[0:64, :])
                        nc.vector.tensor_tensor(
                            out=ctxT[jht][jhr:jhr + 64, ds(jqh * 1024, 1024)],
                            in0=jctx[0:64, :], in1=rbc, op=MULT)

                    def emit_outproj_unit(ti):
                        o_sb = outp.tile([128, HID], F32, name="o_sb")
                        for oc in range(2):
                            o_ps = psS.tile([128, 512], F32, name="o_ps",
                                            tag="s_ps")
                            for hc in range(2):
                                nc.tensor.matmul(
                                    o_ps, lhsT=ctxT[hc][:, ts(ti, 128)],
                                    rhs=wo_sb[:, hc, ts(oc, 512)],
                                    start=(hc == 0), stop=(hc == 1))
                            nc.vector.tensor_copy(o_sb[:, ts(oc, 512)], o_ps)
                        if not os.environ.get("NO_OUT_DMA"):
                            nc.sync.dma_start(out[ts(ti, 128)], o_sb)

                    from collections import deque
                    deferred = deque()
                    pv_q = deque()
                    pv_lag = int(os.environ.get("PV_LAG", "2"))
                    slot = 0
                    due = []        # (due_slot, fn)

                    def emit_pv_pair(job):
                        jpr, jctxA, jctxB, jpt, jkt = job[:5]
                        for hh, jctx, col in ((2 * jpr, jctxA, 0),
                                              (2 * jpr + 1, jctxB, 1)):
                            nc.tensor.matmul(
                                jctx, lhsT=v_sb[hh][:, jkt, :],
                                rhs=jpt[:, ts(col, 512)],
                                start=(jkt == 0), stop=(jkt == NKT - 1))

                    def emit_norm_pair(jqc, jpr, jctxA, jctxB):
                        recip = npool.tile([1, 1024], F32R, name="recip")
                        with nc.allow_low_precision(reason="f32r recip"):
                            nc.vector.reciprocal(recip[:, 0:512],
                                                 jctxA[64:65, :])
                            nc.vector.reciprocal(recip[:, 512:1024],
                                                 jctxB[64:65, :])
                        rbc = npool.tile([64, 1024], F32, name="rbc")
                        bc_ps = psS.tile([128, 1024], F32, name="s_ps",
                                         tag="s_ps")
                        for col in range(2):
                            nc.tensor.matmul(bc_ps[0:64, ts(col, 512)],
                                             lhsT=ones_f32,
                                             rhs=recip[:, ts(col, 512)],
                                             start=True, stop=True)
                        nc.vector.tensor_copy(rbc, bc_ps[0:64, :])
                        for hh, jctx, col in ((2 * jpr, jctxA, 0),
                                              (2 * jpr + 1, jctxB, 1)):
                            nc.vector.tensor_tensor(
                                out=ctxT[jpr][(hh % 2) * 64:(hh % 2) * 64 + 64,
                                              ds(jqc * 512, 512)],
                                in0=jctx[0:64, :], in1=rbc[:, ts(col, 512)],
                                op=MULT)

                    def run_due():
                        for item in [d for d in due if d[0] <= slot]:
                            due.remove(item)
                            item[1]()

                    for qc in range(4):          # 512-token query chunks
                        for pr in range(2):      # head pairs (2pr, 2pr+1)
                            ctxA = psX.tile([65, 512], F32, name="ctx_ps")
                            ctxB = psX.tile([65, 512], F32, name="ctx_ps")
                            for kt in range(NKT):
                                s_ps = psS.tile([128, 1024], F32, name="s_ps")
                                for col in range(2):
                                    hr = col * 64
                                    nc.tensor.matmul(
                                        s_ps[:, ts(col, 512)],
                                        lhsT=k_sb[pr][hr:hr + 64, ts(kt, 128)],
                                        rhs=q_sb[pr][hr:hr + 64,
                                                     ds(qc * 512, 512)],
                                        start=True, stop=True,
                                        tile_position=(hr, 0))
                                pt = ptp.tile([128, 1024], BF16, name="pt")
                                nc.scalar.activation(pt, s_ps, EXP,
                                                     bias=maskb_sb[:, kt:kt + 1],
                                                     scale=0.125)
                                pv_q.append((pr, ctxA, ctxB, pt, kt, qc))
                                if len(pv_q) > pv_lag:
                                    job = pv_q.popleft()
                                    emit_pv_pair(job)
                                    if job[4] == NKT - 1:
                                        jpr, jA, jB, jqc = (job[0], job[1],
                                                            job[2], job[5])
                                        due.append(
                                            (slot + 2, (lambda a=jqc, b=jpr,
                                                        c=jA, d=jB:
                                                        emit_norm_pair(a, b,
                                                                       c, d))))
                                        if jpr == 1:
                                            due.append(
                                                (slot + 3,
                                                 (lambda p=jqc: deferred.extend(
                                                     4 * p + i
                                                     for i in range(4)))))
                                run_due()
                                if deferred and kt % 4 == 2:
                                    emit_outproj_unit(deferred.popleft())
                                slot += 1
                    # tail
                    while pv_q:
                        job = pv_q.popleft()
                        emit_pv_pair(job)
                        if job[4] == NKT - 1:
                            jpr, jA, jB, jqc = (job[0], job[1], job[2],
                                                job[5])
                            due.append((slot, (lambda a=jqc, b=jpr, c=jA,
                                               d=jB:
                                               emit_norm_pair(a, b, c, d))))
                    slot += 1000
                    run_due()
                    for i in range(4):
                        deferred.append(12 + i)
                    while deferred:
                        emit_outproj_unit(deferred.popleft())

    nc.compile()
    return nc


_NC = None


def shard_inputs(x, mask, qkv_w, qkv_b, out_w):
    bf = ml_dtypes.bfloat16
    in_maps = []
    for c in range(N_CORES):
        b, g = c // 4, c % 4
        hs = slice(DH * g, DH * (g + 1))
        xTc = np.ascontiguousarray(x[b].T).astype(bf)
        idx = np.where(mask[b] != 0)[0]
        assert len(idx) <= KC, f"unmasked {len(idx)} > KC={KC}"
        pad = np.zeros(KC - len(idx), np.int64)
        idxp = np.concatenate([idx, pad])
        xTkc = np.ascontiguousarray(x[b][idxp].T).astype(bf)
        mbk = np.full(KC, np.float32(NEG), np.float32)
        mbk[:len(idx)] = 0.0
        mbk = np.ascontiguousarray(mbk.reshape(NKT, 128).T).astype(np.float32)
        wq = np.ascontiguousarray(qkv_w[hs, :].T).astype(bf)
        wk = np.ascontiguousarray(qkv_w[1024 + DH * g:1024 + DH * (g + 1), :].T
                                  ).astype(bf)
        wv = np.ascontiguousarray(qkv_w[2048 + DH * g:2048 + DH * (g + 1), :].T
                                  ).astype(bf)
        qb = np.concatenate([qkv_b[hs], qkv_b[1024 + DH * g:1024 + DH * (g + 1)],
                             qkv_b[2048 + DH * g:2048 + DH * (g + 1)]]
                            )[None, :].astype(bf)
        wo = np.ascontiguousarray(out_w[:, hs].T).astype(bf)
        in_maps.append({"xT": xTc, "xTk": xTkc, "wqT": wq, "wkT": wk,
                        "wvT": wv, "qkvb": qb, "woT": wo, "maskb": mbk})
    return in_maps


def run(in_maps, **kwargs):
    global _NC
    if _NC is None:
        _NC = build_program()
    return bass_utils.run_bass_kernel_spmd(
        _NC, in_maps, core_ids=list(range(N_CORES)), **kwargs)


def kernel(x, mask, qkv_w, qkv_b, out_w, out_b):
    global KC, NKT, _NC
    x = np.asarray(x)
    mask = np.asarray(mask)
    need = int(np.max(np.sum(mask != 0, axis=1)))
    kc = max(128, ((need + 127) // 128) * 128)
    if kc != KC:
        KC, NKT = kc, kc // 128
        _NC = None
    qkv_w = np.asarray(qkv_w)
    qkv_b = np.asarray(qkv_b)
    out_w = np.asarray(out_w)
    out_b = np.asarray(out_b)
    in_maps = shard_inputs(x, mask, qkv_w, qkv_b, out_w)
    res = run(in_maps)
    parts = [r["out"] for r in res.results]
    full = np.empty((2, S, HID), np.float32)
    for b in range(2):
        acc = parts[4 * b].astype(np.float32)
        for g in range(1, 4):
            acc = acc + parts[4 * b + g]
        full[b] = acc + out_b[None, :]
    return full

